# revision 1
# baseline (speedup 1.0000x reference)
"""Causal self-attention (B=2, T=2048, D=2048, 16 heads) on 8 trn2 cores.

Sharding: tensor-parallel over heads — 2 heads per core. Each core computes
q/k/v projections for its 2 heads (column-parallel), causal attention per
head, and a partial output projection (row-parallel). Host sums the 8
partial outputs.

Layout strategy per core (all matmuls contract over the partition dim):
  xT   [D_MODEL, B*T]   (host-pretransposed x)
  wqT  [D_MODEL, 256]   (Wq rows for this core's heads, transposed)
  qT_h [128, B*T]       = WqT_h.T @ xT   (head dim on partitions)
  S.T  [j, i] tiles     = kT_tile.T @ qT_chunk    (keys on partitions)
  PT   = exp(S.T / sqrt(128)) * causal_mask       (ACT, PSUM->SBUF)
  den  [1, i]           = ones.T @ PT   (PE partition-sum, accumulated)
  outT [d, i]           = v_tile.T @ PT (accumulated over j tiles)
  normalize: outT *= broadcast(1/den)   (GpSimd bcast + DVE mult)
  y    [t, m] partial   = outT_tile.T @ WoT_chunk (accum over 2 heads)
"""

import math
from contextlib import ExitStack

import numpy as np
import ml_dtypes

import concourse.bass as bass
import concourse.mybir as mybir
import concourse.tile as tile
from concourse import bacc
from concourse.bass_utils import run_bass_kernel_spmd
from concourse.masks import make_identity

P = 128
D_MODEL = 2048
NUM_HEADS = 16
D = 128            # head dim
B, T = 2, 2048
BT = B * T         # 4096
NCORES = 8
HPC = NUM_HEADS // NCORES   # 2 heads per core
KD = D_MODEL // P           # 16 d_model tiles
TJ = T // P                 # 16 key tiles per batch
IC = 512                    # query chunk width
NI = T // IC                # 4 query chunks per batch
TCH = BT // IC              # 8 token chunks for projections

F32 = mybir.dt.float32

_DT = {
    "f32": mybir.dt.float32,
    "f32r": mybir.dt.float32r,
    "bf16": mybir.dt.bfloat16,
}
_NP = {
    "f32": np.float32,
    "f32r": np.float32,
    "bf16": ml_dtypes.bfloat16,
}

F32R = mybir.dt.float32r


# dtype config: x/w = projection inputs, s = qT/kT storage (S matmul inputs),
# pt = exp'd probabilities, v = value tiles, o = outT storage (outproj lhsT),
# wo = Wo tiles. overlap = double-buffer qkv arrays across heads (more SBUF).
CFG_SAFE = dict(x="f32", w="f32", s="f32", pt="f32", v="f32", o="f32",
                wo="f32", overlap=False)
CFG_FAST = dict(x="bf16", w="bf16", s="bf16", pt="bf16", v="bf16", o="f32r",
                wo="f32r", overlap=True)
# validated: rel err 2.27e-4 vs fp32 reference, ~650 us on HW
CFG_F32R = dict(x="f32r", w="f32r", s="f32r", pt="f32r", v="f32r", o="f32r",
                wo="f32r", overlap=False)


def _emit(tc, cfg, xT, wqT, wkT, wvT, woT, y):
    nc = tc.nc
    x_dt = _DT[cfg["x"]]
    w_dt = _DT[cfg["w"]]
    s_dt = _DT[cfg["s"]]
    pt_dt = _DT[cfg["pt"]]
    v_dt = _DT[cfg["v"]]
    o_dt = _DT[cfg["o"]]
    wo_dt = _DT[cfg["wo"]]
    qb = 2 if cfg["overlap"] else 1   # bufs for per-head qkv arrays

    with ExitStack() as ctx:
        consts = ctx.enter_context(tc.tile_pool(name="consts", bufs=1))
        wpool = ctx.enter_context(tc.tile_pool(name="wpool", bufs=2))
        xpool = ctx.enter_context(tc.tile_pool(name="xpool", bufs=7))
        wopool = ctx.enter_context(tc.tile_pool(name="wopool", bufs=4))
        arrs = ctx.enter_context(tc.tile_pool(name="arrs", bufs=qb))
        arrs2 = ctx.enter_context(tc.tile_pool(name="arrs2", bufs=2))
        ptpool = ctx.enter_context(tc.tile_pool(name="ptpool", bufs=5))
        smalls = ctx.enter_context(tc.tile_pool(name="smalls", bufs=2))
        ypool = ctx.enter_context(tc.tile_pool(name="ypool", bufs=2))
        psum = ctx.enter_context(tc.tile_pool(name="psum", bufs=1, space="PSUM"))
        def _const(shape, dt, tag, fill_fn):
            # gpsimd memset/affine can't write f32r; stage in f32 then copy.
            if dt == F32R:
                stg = consts.tile([P, IC], F32, tag="stg",
                                  name="stg")[:shape[0], :shape[1]]
                fill_fn(stg)
                out = consts.tile(shape, dt, tag=tag, name=tag)
                nc.vector.tensor_copy(out, stg)
                return out
            out = consts.tile(shape, dt, tag=tag, name=tag)
            fill_fn(out)
            return out

        ident = _const([P, P], v_dt, "ident", lambda t: make_identity(nc, t))
        ones_col = _const([P, 1], pt_dt, "ones",
                          lambda t: nc.vector.memset(t, 1.0))

        # tri_mask[p, i] = 1.0 if i >= p else 0 (upper triangular keep)
        def _fill_tri(t):
            nc.gpsimd.memset(t, 0.0)
            nc.gpsimd.affine_select(
                out=t, in_=t, compare_op=mybir.AluOpType.is_gt,
                fill=1.0, base=0, pattern=[[-1, P]], channel_multiplier=1,
            )

        tri_mask = _const([P, P], pt_dt, "trimask", _fill_tri)

        xT3 = xT.rearrange("(ko p) t -> p ko t", p=P)
        w3 = {
            "q": wqT.rearrange("(ko p) o -> p ko o", p=P),
            "k": wkT.rearrange("(ko p) o -> p ko o", p=P),
            "v": wvT.rearrange("(ko p) o -> p ko o", p=P),
        }

        outTs = []
        scale = 1.0 / math.sqrt(D)

        for h in range(HPC):
            # ---- projections for head h: qT/kT [128, BT], vT -> v ----
            w_sb = {}
            for nm in ("q", "k", "v"):
                wt = wpool.tile([P, KD, D], w_dt, tag=f"w{nm}")
                nc.sync.dma_start(wt, w3[nm][:, :, h * D:(h + 1) * D])
                w_sb[nm] = wt
            qT = arrs.tile([P, BT], s_dt, tag="qT")
            kT = arrs.tile([P, BT], s_dt, tag="kT")
            vT = arrs.tile([P, BT], v_dt, tag="vT")
            dests = {"q": qT, "k": kT, "v": vT}
            for tch in range(TCH):
                tsl = slice(tch * IC, (tch + 1) * IC)
                ps = {nm: psum.tile([P, IC], F32, tag=f"s{i}", name=f"s{i}")
                      for i, nm in enumerate(("q", "k", "v"))}
                for kt in range(KD):
                    xt = xpool.tile([P, IC], x_dt, tag="xt")
                    nc.sync.dma_start(xt, xT3[:, kt, tsl])
                    for nm in ("q", "k", "v"):
                        nc.tensor.matmul(
                            ps[nm], w_sb[nm][:, kt], xt,
                            start=(kt == 0), stop=(kt == KD - 1),
                        )
                for nm in ("q", "k", "v"):
                    nc.vector.tensor_copy(dests[nm][:, tsl], ps[nm])

            # transpose vT -> v [128, B, TJ, D] (token tiles on partitions)
            v_sb = arrs.tile([P, B, TJ, D], v_dt, tag="v")
            for b in range(B):
                for jt in range(TJ):
                    pst = psum.tile([P, P], v_dt, tag="s3")
                    nc.tensor.transpose(
                        pst, vT[:, b * T + jt * P: b * T + (jt + 1) * P], ident)
                    nc.vector.tensor_copy(v_sb[:, b, jt], pst)

            # ---- attention for head h ----
            outT = arrs2.tile([P, BT], o_dt, tag="outT")
            outTs.append(outT)
            for b in range(B):
                for ic in range(NI):
                    isl = slice(b * T + ic * IC, b * T + (ic + 1) * IC)
                    nj = ic * 4 + 4          # causal: j tiles 0..nj-1
                    ck = h * B * NI + b * NI + ic
                    ps_o = psum.tile([P, IC], F32, tag=f"o{ck % 2}",
                                     name=f"o{ck % 2}")
                    pt_acc = smalls.tile([P, IC], pt_dt, tag="ptacc")
                    for jt in range(nj):
                        m = jt - ic * 4
                        # partial diagonal tiles: columns < m*128 are fully
                        # masked; restrict all work to the live sub-range.
                        lo = max(m, 0) * P
                        ps_s = psum.tile([P, IC], F32, tag=f"s{jt % 4}",
                                         name=f"s{jt % 4}")
                        nc.tensor.matmul(
                            ps_s[:, lo:],
                            kT[:, b * T + jt * P: b * T + (jt + 1) * P],
                            qT[:, b * T + ic * IC + lo:
                               b * T + (ic + 1) * IC], start=True, stop=True,
                        )
                        pt = ptpool.tile([P, IC], pt_dt, tag="pt")
                        nc.scalar.activation(
                            pt[:, lo:], ps_s[:, lo:],
                            mybir.ActivationFunctionType.Exp, scale=scale)
                        if m >= 0:
                            nc.vector.tensor_tensor(
                                pt[:, lo:lo + P], pt[:, lo:lo + P],
                                tri_mask, mybir.AluOpType.mult)
                        if jt == 0:
                            nc.vector.tensor_copy(pt_acc, pt)
                        else:
                            nc.vector.tensor_tensor(
                                pt_acc[:, lo:], pt_acc[:, lo:], pt[:, lo:],
                                mybir.AluOpType.add)
                        nc.tensor.matmul(
                            ps_o[:, lo:], v_sb[:, b, jt], pt[:, lo:],
                            start=(jt == 0), stop=(jt == nj - 1),
                            skip_group_check=True,
                        )
                    # denominators: one partition-sum matmul per chunk
                    ps_d = psum.tile([1, IC], F32, tag="den", name="den")
                    nc.tensor.matmul(ps_d, ones_col, pt_acc,
                                     start=True, stop=True,
                                     skip_group_check=True)
                    den_sb = smalls.tile([1, IC], F32, tag="densb")
                    nc.vector.tensor_copy(den_sb, ps_d)
                    bc = smalls.tile([P, IC], F32, tag="bc")
                    nc.gpsimd.partition_broadcast(bc, den_sb)
                    rb = smalls.tile([P, IC], F32, tag="rb")
                    nc.vector.reciprocal_approx_fast(out=rb, in_=bc)
                    nc.vector.tensor_tensor(
                        outT[:, isl], ps_o, rb, mybir.AluOpType.mult)

        # ---- output projection: y[t, m] partial over this core's heads ----
        woT3 = woT.rearrange("(h p) m -> h p m", p=P)
        for mc in range(D_MODEL // IC):
            msl = slice(mc * IC, (mc + 1) * IC)
            wo_sb = []
            for h in range(HPC):
                wt = wopool.tile([P, IC], wo_dt, tag="wo", name="wo")
                nc.sync.dma_start(wt, woT3[h, :, msl])
                wo_sb.append(wt)
            for tt in range(BT // P):
                ps_y = psum.tile([P, IC], F32, tag="y")
                for h in range(HPC):
                    nc.tensor.matmul(
                        ps_y, outTs[h][:, tt * P:(tt + 1) * P], wo_sb[h],
                        start=(h == 0), stop=(h == HPC - 1),
                    )
                y_sb = ypool.tile([P, IC], F32, tag="y")
                if mc == 0:
                    nc.vector.tensor_copy(y_sb, ps_y)
                else:
                    nc.scalar.copy(y_sb, ps_y)
                nc.sync.dma_start(y[tt * P:(tt + 1) * P, msl], y_sb)


def _build(cfg):
    nc = bacc.Bacc("TRN2", target_bir_lowering=False, debug=False,
                   num_devices=NCORES)
    xT = nc.dram_tensor("xT", [D_MODEL, BT], _DT[cfg["x"]],
                        kind="ExternalInput").ap()
    wqT = nc.dram_tensor("wqT", [D_MODEL, HPC * D], _DT[cfg["w"]],
                         kind="ExternalInput").ap()
    wkT = nc.dram_tensor("wkT", [D_MODEL, HPC * D], _DT[cfg["w"]],
                         kind="ExternalInput").ap()
    wvT = nc.dram_tensor("wvT", [D_MODEL, HPC * D], _DT[cfg["w"]],
                         kind="ExternalInput").ap()
    woT = nc.dram_tensor("woT", [HPC * D, D_MODEL], _DT[cfg["wo"]],
                         kind="ExternalInput").ap()
    y = nc.dram_tensor("y", [BT, D_MODEL], F32, kind="ExternalOutput").ap()
    with tile.TileContext(nc) as tc:
        _emit(tc, cfg, xT, wqT, wkT, wvT, woT, y)
    nc.compile()
    return nc


def _prep_inputs(x, Wq, Wk, Wv, Wo, cfg):
    xnp = _NP[cfg["x"]]
    wnp = _NP[cfg["w"]]
    wonp = _NP[cfg["wo"]]
    xT = np.ascontiguousarray(
        np.asarray(x, np.float32).reshape(BT, D_MODEL).T).astype(xnp)
    in_maps = []
    for c in range(NCORES):
        rows = slice(c * HPC * D, (c + 1) * HPC * D)
        in_maps.append({
            "xT": xT,
            "wqT": np.ascontiguousarray(np.asarray(Wq)[rows].T).astype(wnp),
            "wkT": np.ascontiguousarray(np.asarray(Wk)[rows].T).astype(wnp),
            "wvT": np.ascontiguousarray(np.asarray(Wv)[rows].T).astype(wnp),
            "woT": np.ascontiguousarray(
                np.asarray(Wo)[:, rows].T).astype(wonp),
        })
    return in_maps


def run(x, Wq, Wk, Wv, Wo, cfg=None, trace=False):
    cfg = cfg or CFG_F32R
    nc = _build(cfg)
    in_maps = _prep_inputs(x, Wq, Wk, Wv, Wo, cfg)
    try:
        res = run_bass_kernel_spmd(nc, in_maps, core_ids=list(range(NCORES)),
                                   trace=trace)
    except Exception:
        res = run_bass_kernel_spmd(nc, in_maps, core_ids=list(range(NCORES)),
                                   trace=trace)
    y = np.zeros((BT, D_MODEL), np.float32)
    for r in res.results:
        y += r["y"]
    return y.reshape(B, T, D_MODEL), res


def kernel(x, Wq, Wk, Wv, Wo):
    y, _ = run(x, Wq, Wk, Wv, Wo)
    return y



# revision 5
# speedup vs baseline: 1.4467x; 1.4467x over previous
"""Causal self-attention (B=2, T=2048, D=2048, 16 heads) on 8 trn2 cores.

Sharding: tensor-parallel over heads - 2 heads per core. Each core computes
q/k/v projections for its 2 heads (column-parallel), causal attention per
head, and a partial output projection (row-parallel). Host sums the 8
partial outputs.

v2 structure (PE-continuity focused, bf16):
  Phase P (projections): stream x token-chunks (512 tokens) ONCE; per chunk
    run all 6 matmul groups (2 heads x q/k/v, 16 kt each) into 6 dedicated
    PSUM banks; PSUM->SBUF casts alternate Scalar/Vector engines. v token
    tiles are PE-transposed one chunk behind, interleaved into the stream.
  Phase A (attention): chunk-pairs (both heads, same (b,ic)) with 1-step
    S-matmul lookahead so exp latency (Scalar) never stalls the PE long.
    S banks rotate x4 (shared with the denominator matmul); PV output banks
    rotate x3; softmax normalize (den copy, reciprocal, partition-broadcast,
    multiply) trails off the critical path.
  Phase O (out projection): tt-outer / mc-inner; outT tile is the stationary
    operand, streaming 4x512 wo columns into 4 PSUM banks; PSUM->SBUF
    copies alternate engines; y emitted bf16 (host sums partials in f32).
"""

import math
from contextlib import ExitStack

import numpy as np
import ml_dtypes

import concourse.bass as bass
import concourse.mybir as mybir
import concourse.tile as tile
from concourse import bacc
from concourse.bass_utils import run_bass_kernel_spmd
from concourse.masks import make_identity

P = 128
D_MODEL = 2048
NUM_HEADS = 16
D = 128            # head dim
B, T = 2, 2048
BT = B * T         # 4096
NCORES = 8
HPC = NUM_HEADS // NCORES   # 2 heads per core
KD = D_MODEL // P           # 16 d_model tiles
TJ = T // P                 # 16 key tiles per batch
IC = 512                    # query chunk width
NI = T // IC                # 4 query chunks per batch
TCH = BT // IC              # 8 token chunks for projections
MC = D_MODEL // IC          # 4 outproj column chunks
TT = BT // P                # 32 token tiles

F32 = mybir.dt.float32
BF16 = mybir.dt.bfloat16

_DT = {"f32": mybir.dt.float32, "bf16": mybir.dt.bfloat16}
_NP = {"f32": np.float32, "bf16": ml_dtypes.bfloat16}

# dtype knobs kept for experimentation; bf16 everywhere is validated.
CFG_FAST = dict(x="bf16", w="bf16", s="bf16", pt="bf16", v="bf16", o="bf16",
                wo="bf16", y="bf16")
CFG_SAFE = dict(CFG_FAST)
CFG_F32R = dict(CFG_FAST)


def _emit(tc, cfg, xT, wqT, wkT, wvT, woT, y):
    nc = tc.nc
    x_dt = _DT[cfg["x"]]
    w_dt = _DT[cfg["w"]]
    s_dt = _DT[cfg["s"]]
    pt_dt = _DT[cfg["pt"]]
    v_dt = _DT[cfg["v"]]
    o_dt = _DT[cfg["o"]]
    wo_dt = _DT[cfg["wo"]]
    y_dt = _DT[cfg["y"]]

    with ExitStack() as ctx:
        consts = ctx.enter_context(tc.tile_pool(name="consts", bufs=1))
        wpool = ctx.enter_context(tc.tile_pool(name="wpool", bufs=1))
        xpool = ctx.enter_context(tc.tile_pool(name="xpool", bufs=3))
        arrs = ctx.enter_context(tc.tile_pool(name="arrs", bufs=1))
        vtpool = ctx.enter_context(tc.tile_pool(name="vtpool", bufs=2))
        ptpool = ctx.enter_context(tc.tile_pool(name="ptpool", bufs=8))
        accpool = ctx.enter_context(tc.tile_pool(name="accpool", bufs=2))
        smalls = ctx.enter_context(tc.tile_pool(name="smalls", bufs=2))
        ypool = ctx.enter_context(tc.tile_pool(name="ypool", bufs=4))
        psum = ctx.enter_context(tc.tile_pool(name="psum", bufs=1,
                                              space="PSUM"))

        ident = consts.tile([P, P], v_dt, tag="ident", name="ident")
        make_identity(nc, ident)
        ones_col = consts.tile([P, 1], pt_dt, tag="ones", name="ones")
        nc.vector.memset(ones_col, 1.0)

        # tri_mask[p, i] = 1.0 if i >= p else 0 (upper triangular keep)
        tri_mask = consts.tile([P, P], pt_dt, tag="trimask", name="trimask")
        nc.gpsimd.memset(tri_mask, 0.0)
        nc.gpsimd.affine_select(
            out=tri_mask, in_=tri_mask, compare_op=mybir.AluOpType.is_gt,
            fill=1.0, base=0, pattern=[[-1, P]], channel_multiplier=1,
        )

        xT3 = xT.rearrange("(ko p) t -> p ko t", p=P)
        w3 = {
            "q": wqT.rearrange("(ko p) o -> p ko o", p=P),
            "k": wkT.rearrange("(ko p) o -> p ko o", p=P),
            "v": wvT.rearrange("(ko p) o -> p ko o", p=P),
        }
        woT3 = woT.rearrange("(h p) m -> h p m", p=P)

        # ---- weight DMAs (split along ko so they spread across queues) ----
        w_sb = {}
        for h in range(HPC):
            for nm in ("q", "k", "v"):
                wt = wpool.tile([P, KD, D], w_dt, tag=f"w{nm}{h}",
                                name=f"w{nm}{h}")
                for part in range(4):
                    ksl = slice(part * 4, (part + 1) * 4)
                    nc.sync.dma_start(
                        wt[:, ksl],
                        w3[nm][:, ksl, h * D:(h + 1) * D])
                w_sb[(h, nm)] = wt
        wo_sb = []
        for h in range(HPC):
            wt = wpool.tile([P, D_MODEL], wo_dt, tag=f"wo{h}", name=f"wo{h}")
            for part in range(4):
                msl = slice(part * IC, (part + 1) * IC)
                nc.sync.dma_start(wt[:, msl], woT3[h, :, msl])
            wo_sb.append(wt)

        # ---- phase P: projections + v transposes ----
        qT = [arrs.tile([P, BT], s_dt, tag=f"qT{h}", name=f"qT{h}") for h in range(HPC)]
        kT = [arrs.tile([P, BT], s_dt, tag=f"kT{h}", name=f"kT{h}") for h in range(HPC)]
        v_sb = [arrs.tile([P, B, TJ, D], v_dt, tag=f"v{h}", name=f"v{h}")
                for h in range(HPC)]
        outT = [arrs.tile([P, BT], o_dt, tag=f"outT{h}", name=f"outT{h}") for h in range(HPC)]

        GROUPS = [(h, nm) for h in range(HPC) for nm in ("q", "k", "v")]
        # per-chunk vt tiles awaiting transpose: list of (vt_tile, tch)
        pend_vt = []

        def emit_transposes(items):
            # 8 transposes (2 heads x 4 token tiles) for one chunk
            for idx, (h, vt, tch) in enumerate(items):
                for sub in range(4):
                    tok = tch * 4 + sub          # global token tile
                    b, jt = divmod(tok, TJ)
                    pst = psum.tile([P, P], v_dt, tag=f"t{6 + (sub % 2)}",
                                    name="vtr")
                    nc.tensor.transpose(pst, vt[:, sub * P:(sub + 1) * P],
                                        ident)
                    eng = nc.vector if sub % 2 else nc.scalar
                    if eng is nc.scalar:
                        nc.scalar.copy(v_sb[h][:, b, jt], pst)
                    else:
                        nc.vector.tensor_copy(v_sb[h][:, b, jt], pst)

        xts = []
        for tch in range(min(2, TCH)):
            xt = xpool.tile([P, KD, IC], x_dt, tag="xt", name="xt")
            for kt in range(KD):
                nc.sync.dma_start(xt[:, kt], xT3[:, kt,
                                  tch * IC:(tch + 1) * IC])
            xts.append(xt)

        for tch in range(TCH):
            xt = xts[tch]
            if tch + 2 < TCH:
                nxt = xpool.tile([P, KD, IC], x_dt, tag="xt", name="xt")
                for kt in range(KD):
                    nc.sync.dma_start(
                        nxt[:, kt],
                        xT3[:, kt, (tch + 2) * IC:(tch + 3) * IC])
                xts.append(nxt)
            tsl = slice(tch * IC, (tch + 1) * IC)
            vt_items = []
            for gi, (h, nm) in enumerate(GROUPS):
                ps = psum.tile([P, IC], F32, tag=f"t{gi}", name=f"p{nm}{h}")
                for kt in range(KD):
                    nc.tensor.matmul(ps, w_sb[(h, nm)][:, kt], xt[:, kt],
                                     start=(kt == 0), stop=(kt == KD - 1))
                if nm == "q":
                    dst = qT[h][:, tsl]
                elif nm == "k":
                    dst = kT[h][:, tsl]
                else:
                    dst = vtpool.tile([P, IC], v_dt, tag=f"vt{h}", name=f"vt{h}")
                    vt_items.append((h, dst, tch))
                    dst = dst[:, :]
                if gi % 2 == 0:
                    nc.vector.tensor_copy(dst, ps)
                else:
                    nc.scalar.copy(dst, ps)
                # interleave last chunk's transposes after the first group
                if gi == 0 and pend_vt:
                    emit_transposes(pend_vt)
                    pend_vt = []
            pend_vt = vt_items
        emit_transposes(pend_vt)

        # ---- phase A: attention, chunk-pairs over heads ----
        scale = 1.0 / math.sqrt(D)
        s_rot = [0]          # rotation over psum tags t0..t3

        def s_tile(shape):
            t = psum.tile(shape, F32, tag=f"t{s_rot[0] % 4}",
                          name=f"s{s_rot[0] % 4}")
            s_rot[0] += 1
            return t

        o_rot = [0]          # rotation over psum tags t4..t6 wait: t6/t7 used
        # PV banks rotate over t4, t5, t6 (t6/t7 only used by transposes in
        # phase P, safe to reuse here; t7 reserved for nothing -> use 4-rot)
        def o_tile():
            t = psum.tile([P, IC], F32, tag=f"t{4 + o_rot[0] % 3}",
                          name=f"o{o_rot[0] % 3}")
            o_rot[0] += 1
            return t

        for b in range(B):
            for ic in range(NI):
                nj = ic * 4 + 4
                isl = slice(b * T + ic * IC, b * T + (ic + 1) * IC)

                ps_o = [o_tile() for _ in range(HPC)]
                pt_acc = [accpool.tile([P, IC], pt_dt, tag=f"acc{h}", name=f"acc{h}")
                          for h in range(HPC)]
                # per-head pending S psum tiles and pt tiles
                ps_s = [[None] * nj for _ in range(HPC)]
                pts = [[None] * nj for _ in range(HPC)]

                def lo_of(jt):
                    m = jt - ic * 4
                    return max(m, 0) * P

                def emit_S(h, jt):
                    lo = lo_of(jt)
                    ps = s_tile([P, IC])
                    nc.tensor.matmul(
                        ps[:, lo:],
                        kT[h][:, b * T + jt * P: b * T + (jt + 1) * P],
                        qT[h][:, b * T + ic * IC + lo:
                              b * T + (ic + 1) * IC],
                        start=True, stop=True)
                    ps_s[h][jt] = ps

                def emit_exp(h, jt):
                    lo = lo_of(jt)
                    m = jt - ic * 4
                    pt = ptpool.tile([P, IC], pt_dt, tag="pt", name="pt")
                    nc.scalar.activation(
                        pt[:, lo:], ps_s[h][jt][:, lo:],
                        mybir.ActivationFunctionType.Exp, scale=scale)
                    if m >= 0:
                        nc.vector.tensor_tensor(
                            pt[:, lo:lo + P], pt[:, lo:lo + P],
                            tri_mask, mybir.AluOpType.mult)
                    if jt == 0:
                        nc.vector.tensor_copy(pt_acc[h], pt)
                    else:
                        nc.vector.tensor_tensor(
                            pt_acc[h][:, lo:], pt_acc[h][:, lo:],
                            pt[:, lo:], mybir.AluOpType.add)
                    pts[h][jt] = pt

                def emit_PV(h, jt):
                    lo = lo_of(jt)
                    nc.tensor.matmul(
                        ps_o[h][:, lo:], v_sb[h][:, b, jt],
                        pts[h][jt][:, lo:],
                        start=(jt == 0), stop=(jt == nj - 1),
                        skip_group_check=True)

                # software-pipelined emission with 1-step S lookahead
                for h in range(HPC):
                    emit_S(h, 0)
                for h in range(HPC):
                    emit_exp(h, 0)
                for jt in range(1, nj):
                    for h in range(HPC):
                        emit_S(h, jt)
                    for h in range(HPC):
                        emit_exp(h, jt)
                    for h in range(HPC):
                        emit_PV(h, jt - 1)
                for h in range(HPC):
                    emit_PV(h, nj - 1)

                # denominators + normalize (trailing, off critical path)
                for h in range(HPC):
                    ps_d = s_tile([1, IC])
                    nc.tensor.matmul(ps_d, ones_col, pt_acc[h],
                                     start=True, stop=True,
                                     skip_group_check=True)
                    den_sb = smalls.tile([1, IC], F32, tag=f"den{h}", name=f"den{h}")
                    nc.vector.tensor_copy(den_sb, ps_d)
                    rb1 = smalls.tile([1, IC], F32, tag=f"rb1{h}", name=f"rb1{h}")
                    nc.vector.reciprocal_approx_fast(out=rb1, in_=den_sb)
                    bc = smalls.tile([P, IC], F32, tag=f"bc{h}", name=f"bc{h}")
                    nc.gpsimd.partition_broadcast(bc, rb1)
                    nc.vector.tensor_tensor(
                        outT[h][:, isl], ps_o[h], bc, mybir.AluOpType.mult)

        # ---- phase O: output projection ----
        for tt in range(TT):
            ps_y = [psum.tile([P, IC], F32, tag=f"t{mc}", name=f"y{mc}")
                    for mc in range(MC)]
            for h in range(HPC):
                for mc in range(MC):
                    nc.tensor.matmul(
                        ps_y[mc], outT[h][:, tt * P:(tt + 1) * P],
                        wo_sb[h][:, mc * IC:(mc + 1) * IC],
                        start=(h == 0), stop=(h == HPC - 1))
            for mc in range(MC):
                y_sb = ypool.tile([P, IC], y_dt, tag="y", name="ysb")
                if mc % 2 == 0:
                    nc.vector.tensor_copy(y_sb, ps_y[mc])
                else:
                    nc.scalar.copy(y_sb, ps_y[mc])
                nc.sync.dma_start(
                    y[tt * P:(tt + 1) * P, mc * IC:(mc + 1) * IC], y_sb)


def _build(cfg):
    nc = bacc.Bacc("TRN2", target_bir_lowering=False, debug=False,
                   num_devices=NCORES)
    xT = nc.dram_tensor("xT", [D_MODEL, BT], _DT[cfg["x"]],
                        kind="ExternalInput").ap()
    wqT = nc.dram_tensor("wqT", [D_MODEL, HPC * D], _DT[cfg["w"]],
                         kind="ExternalInput").ap()
    wkT = nc.dram_tensor("wkT", [D_MODEL, HPC * D], _DT[cfg["w"]],
                         kind="ExternalInput").ap()
    wvT = nc.dram_tensor("wvT", [D_MODEL, HPC * D], _DT[cfg["w"]],
                         kind="ExternalInput").ap()
    woT = nc.dram_tensor("woT", [HPC * D, D_MODEL], _DT[cfg["wo"]],
                         kind="ExternalInput").ap()
    y = nc.dram_tensor("y", [BT, D_MODEL], _DT[cfg["y"]],
                       kind="ExternalOutput").ap()
    with tile.TileContext(nc) as tc:
        _emit(tc, cfg, xT, wqT, wkT, wvT, woT, y)
    nc.compile()
    return nc


def _prep_inputs(x, Wq, Wk, Wv, Wo, cfg):
    xnp = _NP[cfg["x"]]
    wnp = _NP[cfg["w"]]
    wonp = _NP[cfg["wo"]]
    xT = np.ascontiguousarray(
        np.asarray(x, np.float32).reshape(BT, D_MODEL).T).astype(xnp)
    in_maps = []
    for c in range(NCORES):
        rows = slice(c * HPC * D, (c + 1) * HPC * D)
        in_maps.append({
            "xT": xT,
            "wqT": np.ascontiguousarray(np.asarray(Wq)[rows].T).astype(wnp),
            "wkT": np.ascontiguousarray(np.asarray(Wk)[rows].T).astype(wnp),
            "wvT": np.ascontiguousarray(np.asarray(Wv)[rows].T).astype(wnp),
            "woT": np.ascontiguousarray(
                np.asarray(Wo)[:, rows].T).astype(wonp),
        })
    return in_maps


def run(x, Wq, Wk, Wv, Wo, cfg=None, trace=False):
    cfg = cfg or CFG_FAST
    nc = _build(cfg)
    in_maps = _prep_inputs(x, Wq, Wk, Wv, Wo, cfg)
    try:
        res = run_bass_kernel_spmd(nc, in_maps, core_ids=list(range(NCORES)),
                                   trace=trace)
    except Exception:
        res = run_bass_kernel_spmd(nc, in_maps, core_ids=list(range(NCORES)),
                                   trace=trace)
    y = np.zeros((BT, D_MODEL), np.float32)
    for r in res.results:
        y += np.asarray(r["y"], dtype=np.float32)
    return y.reshape(B, T, D_MODEL), res


def kernel(x, Wq, Wk, Wv, Wo):
    y, _ = run(x, Wq, Wk, Wv, Wo)
    return y
